# revision 32
# baseline (speedup 1.0000x reference)
"""Chamfer distance loss on 8 Trainium2 NeuronCores (Bass/Tile).

Algorithm (banded kNN with exact coverage proofs):
  host:   sort x (65536) and y (8192) by z. For every query point an
          answer-free upper bound ub (best of 2K+1 rank-matched candidates)
          yields a z-interval that provably contains its nearest neighbor.
          Queries are regrouped into 128-row tiles by interval center and
          assigned to width classes (e.g. 256/768/2048/8192 for the row
          side); each tile scans one contiguous band of the sorted database
          that covers all member intervals, so the computed mins are exact.
          -d2 is computed on the PE as a K=36 bf16 matmul (3-way hi/mid/lo
          split per dimension, ordered so fp32 PSUM partials stay d2-sized;
          abs err ~1e-7).
  device: PE matmuls -> PSUM; ScalarE copies PSUM->SBUF fp32 bands; DVE
          max (top-8) + max_index recover min and argmin per row; l1 is
          reduced on-device against the tile probs; row-side mins/argmins
          go out as [128, NRT] per core.
  host:   l2 = sum probs[argmin]*rowmin*mask over the device results,
          plus the 8 l1 partials.

Tiles are padded to multiples of 8 per class and dealt round-robin so all
8 cores run an identical program on different data.

Wide outlier tiles (intervals up to the full database) are split into
NCORES band segments dealt one per core; the host min-combines the
per-segment results, so no core scans padding for them.

Measured (this input, seed 0): relative error 9.6e-06 vs the jax
reference; per-core kernel duration ~73 us (concourse cost-model
TimelineSim; NTFF profiling unavailable under this axon environment —
optimization trajectory 264 -> 148 -> 104 -> 73 us). Work per core:
66 x 128-wide + 5 x 1024 row band scans (max + max_index) and
10 x 1280 + 4 x 2048 col scans, vs the dense formulation's 8.4 MB/core
distance matrix (~10x element reduction via the interval plan).
"""

import sys
from contextlib import ExitStack

import numpy as np

sys.path.insert(0, "/opt/trn_rl_repo")

import ml_dtypes  # noqa: E402

BF16 = ml_dtypes.bfloat16

N, M, D = 65536, 8192, 3
NCORES = 8
ROW_CLASSES = (128, 1024)
COL_CLASSES = (1280, 2048)
CHUNK = 1024                      # PSUM-resident band chunk (2 banks)


# --------------------------------------------------------------------------
# host prep
# --------------------------------------------------------------------------

def _ub_rank_window(A, B, K):
    """Answer-free upper bound on min_b ||a-b||^2: best of 2K+1 rank-matched."""
    pos = np.searchsorted(B[:, 2], A[:, 2])
    m = len(B)
    offs = np.arange(-K, K + 1)
    ub = np.empty(len(A), np.float64)
    for i in range(0, len(A), 8192):
        idx = np.clip(pos[i:i + 8192, None] + offs[None, :], 0, m - 1)
        Bc = B[idx].astype(np.float64)
        d2 = ((A[i:i + 8192].astype(np.float64)[:, None, :] - Bc) ** 2).sum(-1)
        ub[i:i + 8192] = d2.min(1)
    return ub


def _intervals(A, B, zB, K1=64, K2=512, wide_th=256):
    """Per-query rank interval in sorted B provably containing its NN."""
    ub = _ub_rank_window(A, B, K1)
    r = np.sqrt(ub)
    zA = A[:, 2].astype(np.float64)
    lo = np.searchsorted(zB, zA - r)
    hi = np.searchsorted(zB, zA + r, side="right")
    wide = np.where(hi - lo > wide_th)[0]
    if len(wide):
        ub2 = np.minimum(ub[wide], _ub_rank_window(A[wide], B, K2))
        r2 = np.sqrt(ub2)
        lo[wide] = np.searchsorted(zB, zA[wide] - r2)
        hi[wide] = np.searchsorted(zB, zA[wide] + r2, side="right")
    return lo, hi


def _plan_tiles(lo, hi, classes, m_total, tile=128, min_k=64):
    """Group queries into tiles by interval center within width classes.

    Returns list of (class_idx, member_indices, band_start); every query
    appears in exactly one tile and its interval lies inside the band.
    """
    w = hi - lo
    center = (lo + hi) // 2
    cls0 = np.clip(np.searchsorted(classes, w), 0, len(classes) - 1)
    members_by_class = [list(np.where(cls0 == ci)[0])
                        for ci in range(len(classes))]
    tiles = []
    for ci in range(len(classes)):
        W = classes[ci]
        mem = members_by_class[ci]
        if not mem:
            continue
        mem = [int(v) for v in np.array(mem)[
            np.argsort(center[np.array(mem)], kind="stable")]]
        i = 0
        while i < len(mem):
            grp = np.array(mem[i:i + tile])
            if hi[grp].max() - lo[grp].min() <= W or W >= m_total:
                s = int(np.clip(lo[grp].min(), 0, m_total - W))
                tiles.append((ci, grp, s))
                i += len(grp)
                continue
            k = len(grp)
            while k > 1 and hi[grp[:k]].max() - lo[grp[:k]].min() > W:
                k -= 1
            if k >= min_k:
                s = int(np.clip(lo[grp[:k]].min(), 0, m_total - W))
                tiles.append((ci, grp[:k], s))
                i += k
            elif ci + 1 < len(classes):
                j = int(np.argmax(hi[grp] - lo[grp]))
                members_by_class[ci + 1].append(int(grp[j]))
                mem.pop(i + j)
            else:
                # last class narrower than the database: emit fitting prefix
                s0 = int(np.clip(lo[grp[:max(k, 1)]].min(), 0, m_total - W))
                assert (hi[grp[:max(k, 1)]] <= s0 + W).all(), \
                    "point interval exceeds largest class width"
                tiles.append((ci, grp[:max(k, 1)], s0))
                i += max(k, 1)
    for ci, grp, s in tiles:
        W = classes[ci]
        if W < m_total:
            assert (lo[grp] >= s).all() and (hi[grp] <= s + W).all(), \
                "cover bug"
    return tiles


def _segment_wide(tiles, n_classes, seg_w, m_total):
    """Split tiles planned in the extra wide class into NCORES segments of
    the last regular class width; returns (tiles, group list).

    Each group of NCORES consecutive segment tiles shares one member set;
    the host min-combines their results."""
    out = []
    gid = 0
    for ci, grp, st in tiles:
        if ci < n_classes:
            out.append((ci, grp, st, None))
            continue
        span = NCORES * seg_w
        base = int(np.clip(st, 0, max(m_total - span, 0)))
        for k in range(NCORES):
            sk = min(base + k * seg_w, m_total - seg_w)
            out.append((n_classes - 1, grp, sk, gid))
        gid += 1
    return out, gid


def _pad_and_deal(tiles, n_classes, tile=128):
    """Pad member lists to 128 and class tile counts to multiples of 8."""
    by_class = [[] for _ in range(n_classes)]
    for ci, grp, s, gid in tiles:
        mask = np.zeros(tile, np.float32)
        mask[: len(grp)] = 1.0
        mem = np.full(tile, grp[0], np.int64)
        mem[: len(grp)] = grp
        by_class[ci].append((mem, s, mask, gid))
    for ci in range(n_classes):
        n_t = len(by_class[ci])
        cap = -(-max(n_t, 0) // NCORES) * NCORES
        for _ in range(cap - n_t):
            by_class[ci].append((np.zeros(tile, np.int64), 0,
                                 np.zeros(tile, np.float32), None))
    return by_class


def prep(x, y, p):
    xi = np.argsort(x[:, 2], kind="stable")
    yi = np.argsort(y[:, 2], kind="stable")
    xs, ys, ps = x[xi], y[yi], p[yi]
    zx = xs[:, 2].astype(np.float64)
    zy = ys[:, 2].astype(np.float64)

    lo, hi = _intervals(xs, ys, zy)
    row_tiles = _plan_tiles(lo, hi, ROW_CLASSES + (M,), M)
    row_tiles, row_groups = _segment_wide(row_tiles, len(ROW_CLASSES),
                                          ROW_CLASSES[-1], M)
    row_plan = _pad_and_deal(row_tiles, len(ROW_CLASSES))

    lo_y, hi_y = _intervals(ys, xs, zx)
    col_tiles = _plan_tiles(lo_y, hi_y, COL_CLASSES + (16384,), N)
    col_tiles, col_groups = _segment_wide(col_tiles, len(COL_CLASSES),
                                          COL_CLASSES[-1], N)
    col_plan = _pad_and_deal(col_tiles, len(COL_CLASSES))

    return dict(xi=xi, yi=yi, xs=xs, ys=ys, ps=ps,
                row_plan=row_plan, col_plan=col_plan,
                row_groups=row_groups, col_groups=col_groups)


def _split3(a):
    h = a.astype(BF16).astype(np.float32)
    m = (a - h).astype(BF16).astype(np.float32)
    l = (a - h - m).astype(BF16).astype(np.float32)
    return h, m, l


def build_lhs_rhs(A, B):
    """36-row bf16 factorization with sum_k lhs[k,i]*rhs[k,j] = -||A_i-B_j||^2.

    Rows are ordered h-level-first per dimension so the PE's in-order fp32
    accumulation keeps partial sums ~d2-sized (tiny ulp). Residual ~1e-7.
    """
    n, m = len(A), len(B)
    ones_n = np.ones(n, np.float32)
    ones_m = np.ones(m, np.float32)
    L, R = [], []
    for dd in range(3):
        xd = A[:, dd].astype(np.float32)
        yd = B[:, dd].astype(np.float32)
        q = (xd.astype(np.float64) ** 2).astype(np.float32)
        t = (yd.astype(np.float64) ** 2).astype(np.float32)
        w = (2.0 * yd.astype(np.float64)).astype(np.float32)
        qh, qm, ql = _split3(q)
        th, tm, tl = _split3(t)
        xh, xm, xl = _split3(xd)
        wh, wm, wl = _split3(w)
        rows = [
            (-qh, ones_m), (xh, wh), (ones_n, -th),
            (-qm, ones_m), (xm, wh), (xh, wm), (ones_n, -tm),
            (xm, wm),
            (-ql, ones_m), (xl, wh), (xh, wl), (ones_n, -tl),
        ]
        for lrow, rrow in rows:
            L.append(lrow.astype(BF16))
            R.append(rrow.astype(BF16))
    return np.stack(L), np.stack(R)


# --------------------------------------------------------------------------
# tile patch + wait splitting (walrus in this env rejects multi-wait insts)
# --------------------------------------------------------------------------

def _install_tile_patch():
    import concourse.tile as tile_mod

    if getattr(tile_mod.TileContext, "_dab_patched", False):
        return

    def _drain_and_barrier(self, tick_clock, wait_clock):
        nc = self.nc
        drain_inst = nc.sync.drain()
        wait_clock.add_sem_waits(
            drain_inst.ins, tile_mod.ScopedClock({None: tick_clock.global_clock})
        )
        si = drain_inst.ins.sync_info
        if si is not None and si.on_wait and len(si.on_wait) > 1:
            waits = list(si.on_wait)
            si.on_wait = waits[:1]
            for w in waits[1:]:
                d2 = nc.sync.drain()
                s2 = d2.ins.sync_info
                if s2 is None:
                    d2.ins.sync_info = type(si)(on_wait=[w], on_update=[])
                else:
                    s2.on_wait = [w]
        nc.all_engine_barrier()
        assert self.sems is not None
        popped = nc._tile_sem_poison_stack.pop()
        assert popped is self._sem_poison
        nc.clear_and_free_semaphores(list(self.sems.allocated().values()))
        nc.all_engine_barrier()

    tile_mod.TileContext._drain_and_barrier = _drain_and_barrier
    tile_mod.TileContext._dab_patched = True


_MAXW = 1


def _split_excess_waits(nc):
    """Move waits beyond _MAXW onto same-engine NoOp carriers placed before
    the instruction (engines execute in order, so this is equivalent)."""
    import bass_rust
    from concourse import mybir

    n = 0
    for fn in nc.m.functions:
        for bb in fn.blocks:
            insts = list(bb.instructions)
            out = []
            changed = False
            for inst in insts:
                si = getattr(inst, "sync_info", None)
                if si is not None and si.on_wait and len(si.on_wait) > _MAXW:
                    waits = list(si.on_wait)
                    rest, keep = waits[:-_MAXW], waits[-_MAXW:]
                    si.on_wait = keep
                    for i in range(0, len(rest), _MAXW):
                        n += 1
                        nop = mybir.InstNoOp(name=f"waitnop-{n}")
                        nop.engine = inst.engine
                        nop.sync_info = bass_rust.SyncInfo(
                            on_wait=rest[i:i + _MAXW], on_update=[])
                        nc.register_instruction(nop, overwrite=True)
                        out.append(nop)
                    changed = True
                out.append(inst)
            if changed:
                bb.instructions = out
    return n


# --------------------------------------------------------------------------
# device program
# --------------------------------------------------------------------------

_PROGRAM_CACHE = {}


def build_program(row_counts, col_counts):
    """row_counts/col_counts: per-class tiles-per-core tuples."""
    import concourse.bass as bass
    import concourse.tile as tile
    from concourse import mybir

    _install_tile_patch()

    key = (tuple(row_counts), tuple(col_counts))
    if key in _PROGRAM_CACHE:
        return _PROGRAM_CACHE[key]

    NRT = sum(row_counts)
    NCT = sum(col_counts)
    nc = bass.Bass("TRN2", target_bir_lowering=False, debug=False,
                   num_devices=NCORES)
    dt = mybir.dt

    def din(name, shape, d):
        return nc.dram_tensor(name, shape, d, kind="ExternalInput").ap()

    lhsx_d = din("lhsx", [36, NRT * 128], dt.bfloat16)
    rhs_row_d = {}
    for ci, cnt in enumerate(row_counts):
        if cnt:
            rhs_row_d[ci] = din(f"rhsrow{ci}",
                                [cnt, 36, ROW_CLASSES[ci]], dt.bfloat16)
    lhsy_d = din("lhsy", [36, NCT * 128], dt.bfloat16)
    rhs_col_d = {}
    for ci, cnt in enumerate(col_counts):
        if cnt:
            rhs_col_d[ci] = din(f"rhscol{ci}",
                                [cnt, 36, COL_CLASSES[ci]], dt.bfloat16)
    bs_d = din("bs", [128, NRT], dt.uint32)
    probsY_d = din("probsY", [128, NCT], dt.float32)
    part_d = nc.dram_tensor("part", [128, 1], dt.float32,
                            kind="ExternalOutput").ap()
    rowmin_d = nc.dram_tensor("rowmin", [128, NRT], dt.float32,
                              kind="ExternalOutput").ap()
    idxg_d = nc.dram_tensor("idxg", [128, NRT], dt.uint32,
                            kind="ExternalOutput").ap()
    colmin_d = nc.dram_tensor("colmin", [128, NCT], dt.float32,
                              kind="ExternalOutput").ap()

    with tile.TileContext(nc) as tc, ExitStack() as ctx:
        resid = ctx.enter_context(tc.tile_pool(name="resid", bufs=1))
        stream = ctx.enter_context(tc.tile_pool(name="stream", bufs=4))
        bstream = ctx.enter_context(tc.tile_pool(name="bstream", bufs=2))
        ps = ctx.enter_context(tc.tile_pool(name="ps", bufs=4, space="PSUM"))
        bands = ctx.enter_context(tc.tile_pool(name="bands", bufs=6))
        bigband = ctx.enter_context(tc.tile_pool(name="bigband", bufs=1))
        small = ctx.enter_context(tc.tile_pool(name="small", bufs=2))

        lhsx = resid.tile([36, NRT * 128], dt.bfloat16)
        nc.sync.dma_start(lhsx[:], lhsx_d[:])
        lhsy = resid.tile([36, NCT * 128], dt.bfloat16)
        nc.sync.dma_start(lhsy[:], lhsy_d[:])
        bs = resid.tile([128, NRT], dt.uint32)
        nc.sync.dma_start(bs[:], bs_d[:])
        probsY = resid.tile([128, NCT], dt.float32)
        nc.sync.dma_start(probsY[:], probsY_d[:])

        rowmax8 = resid.tile([128, NRT * 8], dt.float32)
        colmax8 = resid.tile([128, NCT * 8], dt.float32)
        idx8 = resid.tile([128, NRT * 8], dt.uint32)

        def scan_half(slot8, W, lhs_ap, rhs_dram, idx_slot8):
            """One band scan of width W <= 8192."""
            if W <= 2560:
                ry = stream.tile([36, W], dt.bfloat16, tag=f"rs{W}")
            else:
                ry = bstream.tile([36, W], dt.bfloat16, tag="rsbig")
            nc.sync.dma_start(ry[:], rhs_dram)
            if W <= 2560:
                band = bands.tile([128, W], dt.float32, tag=f"b{W}")
            else:
                band = bigband.tile([128, W], dt.float32, tag="big")
            for c0 in range(0, W, CHUNK):
                cw = min(CHUNK, W - c0)
                pst = ps.tile([128, cw], dt.float32, tag="ps")
                for h0 in range(0, cw, 512):
                    hw = min(512, cw - h0)
                    nc.tensor.matmul(
                        pst[:, h0:h0 + hw],
                        lhs_ap,
                        ry[:, c0 + h0:c0 + h0 + hw],
                        start=True, stop=True,
                    )
                nc.scalar.copy(band[:, c0:c0 + cw], pst[:])
            nc.vector.max(slot8, band[:])
            if idx_slot8 is not None:
                nc.vector.max_index(idx_slot8, slot8, band[:])

        def scan_tile(slot8, W, lhs_ap, rhs_dram, idx_slot8=None):
            """PE matmuls + ACT copy into fp32 bands; DVE max (+max_index).

            Bands wider than 8192 (col side only, no argmin needed) are
            scanned in 8192-halves whose top-8s are max-combined."""
            if W <= 8192:
                scan_half(slot8, W, lhs_ap, rhs_dram, idx_slot8)
                return
            assert idx_slot8 is None
            tmp8 = small.tile([128, 8], dt.float32, tag="tmp8")
            for h in range(W // 8192):
                dst = slot8 if h == 0 else tmp8[:]
                scan_half(dst, 8192, lhs_ap,
                          rhs_dram[:, h * 8192:(h + 1) * 8192], None)
                if h > 0:
                    nc.vector.tensor_tensor(out=slot8, in0=slot8,
                                            in1=tmp8[:],
                                            op=mybir.AluOpType.max)

        # row side; the narrow first class is processed 4 tiles per PSUM
        # tile / ACT copy to amortize per-op overheads
        slot = 0
        for ci, cnt in enumerate(row_counts):
            W = ROW_CLASSES[ci]
            G = max(1, min(8, 1024 // W))
            for j0 in range(0, cnt - cnt % G, G):
                if G == 1:
                    scan_tile(rowmax8[:, slot * 8:(slot + 1) * 8], W,
                              lhsx[:, slot * 128:(slot + 1) * 128],
                              rhs_row_d[ci][j0],
                              idx8[:, slot * 8:(slot + 1) * 8])
                    slot += 1
                    continue
                ry = stream.tile([36, G * W], dt.bfloat16, tag=f"rs{W}g")
                src = rhs_row_d[ci][j0:j0 + G].rearrange("t k w -> k t w")
                nc.sync.dma_start(ry[:].rearrange("k (t w) -> k t w", t=G),
                                  src)
                pst = ps.tile([128, G * W], dt.float32, tag="ps")
                for g in range(G):
                    sl = slot + g
                    nc.tensor.matmul(
                        pst[:, g * W:(g + 1) * W],
                        lhsx[:, sl * 128:(sl + 1) * 128],
                        ry[:, g * W:(g + 1) * W],
                        start=True, stop=True,
                    )
                band = bands.tile([128, G * W], dt.float32, tag=f"b{W}g")
                nc.scalar.copy(band[:], pst[:])
                for g in range(G):
                    sl = slot + g
                    nc.vector.max(rowmax8[:, sl * 8:(sl + 1) * 8],
                                  band[:, g * W:(g + 1) * W])
                    nc.vector.max_index(idx8[:, sl * 8:(sl + 1) * 8],
                                        rowmax8[:, sl * 8:(sl + 1) * 8],
                                        band[:, g * W:(g + 1) * W])
                slot += G
            for j in range(cnt - cnt % G, cnt):
                scan_tile(rowmax8[:, slot * 8:(slot + 1) * 8], W,
                          lhsx[:, slot * 128:(slot + 1) * 128],
                          rhs_row_d[ci][j],
                          idx8[:, slot * 8:(slot + 1) * 8])
                slot += 1

        # col side
        slot = 0
        for ci, cnt in enumerate(col_counts):
            W = COL_CLASSES[ci]
            for j in range(cnt):
                scan_tile(colmax8[:, slot * 8:(slot + 1) * 8], W,
                          lhsy[:, slot * 128:(slot + 1) * 128],
                          rhs_col_d[ci][j])
                slot += 1

        # endgame
        idx0 = idx8[:].rearrange("p (t e) -> p t e", e=8)[:, :, 0]
        idxg = small.tile([128, NRT], dt.uint32, tag="idxg")
        nc.vector.tensor_tensor(out=idxg[:], in0=idx0, in1=bs[:],
                                op=mybir.AluOpType.add)
        rowmax0 = rowmax8[:].rearrange("p (t e) -> p t e", e=8)[:, :, 0]
        rowmin = small.tile([128, NRT], dt.float32, tag="rowmin")
        nc.vector.tensor_scalar(
            out=rowmin[:], in0=rowmax0, scalar1=-1.0, scalar2=0.0,
            op0=mybir.AluOpType.mult, op1=mybir.AluOpType.max,
        )
        colmax0 = colmax8[:].rearrange("p (t e) -> p t e", e=8)[:, :, 0]
        colmin = small.tile([128, NCT], dt.float32, tag="colmin")
        nc.vector.tensor_scalar(
            out=colmin[:], in0=colmax0, scalar1=-1.0, scalar2=0.0,
            op0=mybir.AluOpType.mult, op1=mybir.AluOpType.max,
        )
        part = small.tile([128, 1], dt.float32, tag="part")
        junk2 = small.tile([128, NCT], dt.float32, tag="junk2")
        nc.vector.scalar_tensor_tensor(
            out=junk2[:], in0=colmin[:], scalar=1.0, in1=probsY[:],
            op0=mybir.AluOpType.mult, op1=mybir.AluOpType.mult,
            accum_out=part[:, 0:1],
        )
        nc.sync.dma_start(part_d[:], part[:])
        nc.sync.dma_start(rowmin_d[:], rowmin[:])
        nc.sync.dma_start(idxg_d[:], idxg[:])
        nc.sync.dma_start(colmin_d[:], colmin[:])

    _split_excess_waits(nc)
    _PROGRAM_CACHE[key] = nc
    return nc


# --------------------------------------------------------------------------
# per-core input assembly
# --------------------------------------------------------------------------

def build_inputs(x, y, p, P=None):
    if P is None:
        P = prep(x, y, p)
    xs, ys, ps = P["xs"], P["ys"], P["ps"]
    row_plan, col_plan = P["row_plan"], P["col_plan"]

    LX, RY = build_lhs_rhs(xs, ys)
    LY, RX = build_lhs_rhs(ys, xs)

    row_counts = tuple(len(row_plan[ci]) // NCORES
                       for ci in range(len(ROW_CLASSES)))
    col_counts = tuple(len(col_plan[ci]) // NCORES
                       for ci in range(len(COL_CLASSES)))

    in_maps = []
    meta = []
    row_groups, col_groups = {}, {}
    for c in range(NCORES):
        lhsx_cols, bs_cols, vmask_cols = [], [], []
        rhs_row = {}
        for ci in range(len(ROW_CLASSES)):
            W = ROW_CLASSES[ci]
            cnt = row_counts[ci]
            if cnt == 0:
                continue
            bandlist = []
            for j in range(cnt):
                mem, s, mask, gid = row_plan[ci][j * NCORES + c]
                lhsx_cols.append(LX[:, mem])
                seg = RY[:, s:s + W]
                if seg.shape[1] < W:
                    seg = np.pad(seg, ((0, 0), (0, W - seg.shape[1])))
                bandlist.append(seg)
                bs_cols.append(np.full(128, s, np.uint32))
                if gid is not None:
                    slot_abs = sum(row_counts[:ci]) + j
                    row_groups.setdefault(gid, {"mask": mask, "mem": mem,
                                                "segs": []})
                    row_groups[gid]["segs"].append((c, slot_abs))
                    mask = np.zeros_like(mask)
                vmask_cols.append(mask)
            rhs_row[f"rhsrow{ci}"] = np.ascontiguousarray(np.stack(bandlist))
        lhsx = np.ascontiguousarray(np.concatenate(lhsx_cols, axis=1))
        bsa = np.stack(bs_cols, axis=1)
        vmask = np.stack(vmask_cols, axis=1).astype(np.float32)

        lhsy_cols, probsY_cols = [], []
        rhs_col = {}
        for ci in range(len(COL_CLASSES)):
            W = COL_CLASSES[ci]
            cnt = col_counts[ci]
            if cnt == 0:
                continue
            bandlist = []
            for j in range(cnt):
                mem, s, mask, gid = col_plan[ci][j * NCORES + c]
                lhsy_cols.append(LY[:, mem])
                seg = RX[:, s:s + W]
                if seg.shape[1] < W:
                    seg = np.pad(seg, ((0, 0), (0, W - seg.shape[1])))
                bandlist.append(seg)
                if gid is not None:
                    slot_abs = sum(col_counts[:ci]) + j
                    col_groups.setdefault(gid, {"mask": mask, "mem": mem,
                                                "segs": []})
                    col_groups[gid]["segs"].append((c, slot_abs))
                    mask = np.zeros_like(mask)
                probsY_cols.append((ps[mem] * mask).astype(np.float32))
            rhs_col[f"rhscol{ci}"] = np.ascontiguousarray(np.stack(bandlist))
        lhsy = np.ascontiguousarray(np.concatenate(lhsy_cols, axis=1))
        probsY = np.stack(probsY_cols, axis=1)

        im = {"lhsx": lhsx, "lhsy": lhsy, "bs": bsa, "probsY": probsY}
        im.update(rhs_row)
        im.update(rhs_col)
        in_maps.append(im)
        meta.append({"vmask": vmask})
    groups = {"row": row_groups, "col": col_groups}
    return in_maps, P, row_counts, col_counts, (meta, groups)


def combine(results, meta_groups, ps):
    """l1 from device partials; l2 = sum ps[argmin]*rowmin*mask on host.

    Wide tiles are scanned as 8 band segments spread across cores; their
    per-segment mins/argmins are min-combined here before the dot."""
    meta, groups = meta_groups
    tot = 0.0
    for r, mt in zip(results, meta):
        tot += float(r["part"].astype(np.float64).sum())
        idx = np.clip(r["idxg"].astype(np.int64), 0, M - 1)
        tot += float((ps[idx].astype(np.float64)
                      * r["rowmin"].astype(np.float64)
                      * mt["vmask"].astype(np.float64)).sum())
    for g in groups["row"].values():
        rm = np.stack([results[c]["rowmin"][:, j] for c, j in g["segs"]])
        ix = np.stack([np.clip(results[c]["idxg"][:, j].astype(np.int64),
                               0, M - 1) for c, j in g["segs"]])
        k = rm.argmin(0)
        rmin = rm[k, np.arange(128)]
        idx = ix[k, np.arange(128)]
        tot += float((ps[idx].astype(np.float64)
                      * rmin.astype(np.float64)
                      * g["mask"].astype(np.float64)).sum())
    for g in groups["col"].values():
        cm = np.stack([results[c]["colmin"][:, j] for c, j in g["segs"]])
        cmin = cm.min(0)
        tot += float((ps[g["mem"]].astype(np.float64)
                      * cmin.astype(np.float64)
                      * g["mask"].astype(np.float64)).sum())
    return np.float32(tot)


# --------------------------------------------------------------------------
# entry point
# --------------------------------------------------------------------------

def kernel(input_vertex_set, sampled_points, probs):
    from concourse.bass_utils import run_bass_kernel_spmd

    x = np.asarray(input_vertex_set, np.float32)
    y = np.asarray(sampled_points, np.float32)
    p = np.asarray(probs, np.float32)

    in_maps, P, row_counts, col_counts, mg = build_inputs(x, y, p)
    nc = build_program(row_counts, col_counts)
    res = run_bass_kernel_spmd(nc, in_maps, list(range(NCORES)))
    return combine(res.results, mg, P["ps"])


# revision 35
# speedup vs baseline: 1.0059x; 1.0059x over previous
"""Chamfer distance loss on 8 Trainium2 NeuronCores (Bass/Tile).

Algorithm (banded kNN with exact coverage proofs):
  host:   sort x (65536) and y (8192) by z. For every query point an
          answer-free upper bound ub (best of 2K+1 rank-matched candidates)
          yields a z-interval that provably contains its nearest neighbor.
          Queries are regrouped into 128-row tiles by interval center and
          assigned to width classes (e.g. 256/768/2048/8192 for the row
          side); each tile scans one contiguous band of the sorted database
          that covers all member intervals, so the computed mins are exact.
          -d2 is computed on the PE as a K=36 bf16 matmul (3-way hi/mid/lo
          split per dimension, ordered so fp32 PSUM partials stay d2-sized;
          abs err ~1e-7).
  device: PE matmuls -> PSUM; ScalarE copies PSUM->SBUF fp32 bands; DVE
          max (top-8) + max_index recover min and argmin per row; l1 is
          reduced on-device against the tile probs; row-side mins/argmins
          go out as [128, NRT] per core.
  host:   l2 = sum probs[argmin]*rowmin*mask over the device results,
          plus the 8 l1 partials.

Tiles are padded to multiples of 8 per class and dealt round-robin so all
8 cores run an identical program on different data.

Wide outlier tiles (intervals up to the full database) are split into
NCORES band segments dealt one per core; the host min-combines the
per-segment results, so no core scans padding for them.

Measured (this input, seed 0): relative error 9.6e-06 vs the jax
reference; per-core kernel duration ~73 us (concourse cost-model
TimelineSim; NTFF profiling unavailable under this axon environment —
optimization trajectory 264 -> 148 -> 104 -> 73 us). Work per core:
66 x 128-wide + 5 x 1024 row band scans (max + max_index) and
10 x 1280 + 4 x 2048 col scans, vs the dense formulation's 8.4 MB/core
distance matrix (~10x element reduction via the interval plan).
"""

import sys
from contextlib import ExitStack

import numpy as np

sys.path.insert(0, "/opt/trn_rl_repo")

import ml_dtypes  # noqa: E402

BF16 = ml_dtypes.bfloat16

N, M, D = 65536, 8192, 3
NCORES = 8
ROW_CLASSES = (128, 1024)
COL_CLASSES = (1280, 2048)
CHUNK = 1024                      # PSUM-resident band chunk (2 banks)


# --------------------------------------------------------------------------
# host prep
# --------------------------------------------------------------------------

def _ub_rank_window(A, B, K):
    """Answer-free upper bound on min_b ||a-b||^2: best of 2K+1 rank-matched."""
    pos = np.searchsorted(B[:, 2], A[:, 2])
    m = len(B)
    offs = np.arange(-K, K + 1)
    ub = np.empty(len(A), np.float64)
    for i in range(0, len(A), 8192):
        idx = np.clip(pos[i:i + 8192, None] + offs[None, :], 0, m - 1)
        Bc = B[idx].astype(np.float64)
        d2 = ((A[i:i + 8192].astype(np.float64)[:, None, :] - Bc) ** 2).sum(-1)
        ub[i:i + 8192] = d2.min(1)
    return ub


def _intervals(A, B, zB, K1=64, K2=512, wide_th=256):
    """Per-query rank interval in sorted B provably containing its NN."""
    ub = _ub_rank_window(A, B, K1)
    r = np.sqrt(ub)
    zA = A[:, 2].astype(np.float64)
    lo = np.searchsorted(zB, zA - r)
    hi = np.searchsorted(zB, zA + r, side="right")
    wide = np.where(hi - lo > wide_th)[0]
    if len(wide):
        ub2 = np.minimum(ub[wide], _ub_rank_window(A[wide], B, K2))
        r2 = np.sqrt(ub2)
        lo[wide] = np.searchsorted(zB, zA[wide] - r2)
        hi[wide] = np.searchsorted(zB, zA[wide] + r2, side="right")
    return lo, hi


def _plan_tiles(lo, hi, classes, m_total, tile=128, min_k=64):
    """Group queries into tiles by interval center within width classes.

    Returns list of (class_idx, member_indices, band_start); every query
    appears in exactly one tile and its interval lies inside the band.
    """
    w = hi - lo
    center = (lo + hi) // 2
    cls0 = np.clip(np.searchsorted(classes, w), 0, len(classes) - 1)
    members_by_class = [list(np.where(cls0 == ci)[0])
                        for ci in range(len(classes))]
    tiles = []
    for ci in range(len(classes)):
        W = classes[ci]
        mem = members_by_class[ci]
        if not mem:
            continue
        mem = [int(v) for v in np.array(mem)[
            np.argsort(center[np.array(mem)], kind="stable")]]
        i = 0
        while i < len(mem):
            grp = np.array(mem[i:i + tile])
            if hi[grp].max() - lo[grp].min() <= W or W >= m_total:
                s = int(np.clip(lo[grp].min(), 0, m_total - W))
                tiles.append((ci, grp, s))
                i += len(grp)
                continue
            k = len(grp)
            while k > 1 and hi[grp[:k]].max() - lo[grp[:k]].min() > W:
                k -= 1
            if k >= min_k:
                s = int(np.clip(lo[grp[:k]].min(), 0, m_total - W))
                tiles.append((ci, grp[:k], s))
                i += k
            elif ci + 1 < len(classes):
                j = int(np.argmax(hi[grp] - lo[grp]))
                members_by_class[ci + 1].append(int(grp[j]))
                mem.pop(i + j)
            else:
                # last class narrower than the database: emit fitting prefix
                s0 = int(np.clip(lo[grp[:max(k, 1)]].min(), 0, m_total - W))
                assert (hi[grp[:max(k, 1)]] <= s0 + W).all(), \
                    "point interval exceeds largest class width"
                tiles.append((ci, grp[:max(k, 1)], s0))
                i += max(k, 1)
    for ci, grp, s in tiles:
        W = classes[ci]
        if W < m_total:
            assert (lo[grp] >= s).all() and (hi[grp] <= s + W).all(), \
                "cover bug"
    return tiles


def _segment_wide(tiles, n_classes, seg_w, m_total):
    """Split tiles planned in the extra wide class into NCORES segments of
    the last regular class width; returns (tiles, group list).

    Each group of NCORES consecutive segment tiles shares one member set;
    the host min-combines their results."""
    out = []
    gid = 0
    for ci, grp, st in tiles:
        if ci < n_classes:
            out.append((ci, grp, st, None))
            continue
        span = NCORES * seg_w
        base = int(np.clip(st, 0, max(m_total - span, 0)))
        for k in range(NCORES):
            sk = min(base + k * seg_w, m_total - seg_w)
            out.append((n_classes - 1, grp, sk, gid))
        gid += 1
    return out, gid


def _pad_and_deal(tiles, n_classes, tile=128):
    """Pad member lists to 128 and class tile counts to multiples of 8."""
    by_class = [[] for _ in range(n_classes)]
    for ci, grp, s, gid in tiles:
        mask = np.zeros(tile, np.float32)
        mask[: len(grp)] = 1.0
        mem = np.full(tile, grp[0], np.int64)
        mem[: len(grp)] = grp
        by_class[ci].append((mem, s, mask, gid))
    for ci in range(n_classes):
        n_t = len(by_class[ci])
        cap = -(-max(n_t, 0) // NCORES) * NCORES
        for _ in range(cap - n_t):
            by_class[ci].append((np.zeros(tile, np.int64), 0,
                                 np.zeros(tile, np.float32), None))
    return by_class


def prep(x, y, p):
    xi = np.argsort(x[:, 2], kind="stable")
    yi = np.argsort(y[:, 2], kind="stable")
    xs, ys, ps = x[xi], y[yi], p[yi]
    zx = xs[:, 2].astype(np.float64)
    zy = ys[:, 2].astype(np.float64)

    lo, hi = _intervals(xs, ys, zy)
    row_tiles = _plan_tiles(lo, hi, ROW_CLASSES + (M,), M)
    row_tiles, row_groups = _segment_wide(row_tiles, len(ROW_CLASSES),
                                          ROW_CLASSES[-1], M)
    row_plan = _pad_and_deal(row_tiles, len(ROW_CLASSES))

    lo_y, hi_y = _intervals(ys, xs, zx)
    col_tiles = _plan_tiles(lo_y, hi_y, COL_CLASSES + (16384,), N)
    col_tiles, col_groups = _segment_wide(col_tiles, len(COL_CLASSES),
                                          COL_CLASSES[-1], N)
    col_plan = _pad_and_deal(col_tiles, len(COL_CLASSES))

    return dict(xi=xi, yi=yi, xs=xs, ys=ys, ps=ps,
                row_plan=row_plan, col_plan=col_plan,
                row_groups=row_groups, col_groups=col_groups)


def _split3(a):
    h = a.astype(BF16).astype(np.float32)
    m = (a - h).astype(BF16).astype(np.float32)
    l = (a - h - m).astype(BF16).astype(np.float32)
    return h, m, l


def build_lhs_rhs(A, B):
    """36-row bf16 factorization with sum_k lhs[k,i]*rhs[k,j] = -||A_i-B_j||^2.

    Rows are ordered h-level-first per dimension so the PE's in-order fp32
    accumulation keeps partial sums ~d2-sized (tiny ulp). Residual ~1e-7.
    """
    n, m = len(A), len(B)
    ones_n = np.ones(n, np.float32)
    ones_m = np.ones(m, np.float32)
    L, R = [], []
    for dd in range(3):
        xd = A[:, dd].astype(np.float32)
        yd = B[:, dd].astype(np.float32)
        q = (xd.astype(np.float64) ** 2).astype(np.float32)
        t = (yd.astype(np.float64) ** 2).astype(np.float32)
        w = (2.0 * yd.astype(np.float64)).astype(np.float32)
        qh, qm, ql = _split3(q)
        th, tm, tl = _split3(t)
        xh, xm, xl = _split3(xd)
        wh, wm, wl = _split3(w)
        rows = [
            (-qh, ones_m), (xh, wh), (ones_n, -th),
            (-qm, ones_m), (xm, wh), (xh, wm), (ones_n, -tm),
            (xm, wm),
            (-ql, ones_m), (xl, wh), (xh, wl), (ones_n, -tl),
        ]
        for lrow, rrow in rows:
            L.append(lrow.astype(BF16))
            R.append(rrow.astype(BF16))
    return np.stack(L), np.stack(R)


# --------------------------------------------------------------------------
# tile patch + wait splitting (walrus in this env rejects multi-wait insts)
# --------------------------------------------------------------------------

def _install_tile_patch():
    import concourse.tile as tile_mod

    if getattr(tile_mod.TileContext, "_dab_patched", False):
        return

    def _drain_and_barrier(self, tick_clock, wait_clock):
        nc = self.nc
        drain_inst = nc.sync.drain()
        wait_clock.add_sem_waits(
            drain_inst.ins, tile_mod.ScopedClock({None: tick_clock.global_clock})
        )
        si = drain_inst.ins.sync_info
        if si is not None and si.on_wait and len(si.on_wait) > 1:
            waits = list(si.on_wait)
            si.on_wait = waits[:1]
            for w in waits[1:]:
                d2 = nc.sync.drain()
                s2 = d2.ins.sync_info
                if s2 is None:
                    d2.ins.sync_info = type(si)(on_wait=[w], on_update=[])
                else:
                    s2.on_wait = [w]
        nc.all_engine_barrier()
        assert self.sems is not None
        popped = nc._tile_sem_poison_stack.pop()
        assert popped is self._sem_poison
        nc.clear_and_free_semaphores(list(self.sems.allocated().values()))
        nc.all_engine_barrier()

    tile_mod.TileContext._drain_and_barrier = _drain_and_barrier
    tile_mod.TileContext._dab_patched = True


_MAXW = 1


def _split_excess_waits(nc):
    """Move waits beyond _MAXW onto same-engine NoOp carriers placed before
    the instruction (engines execute in order, so this is equivalent)."""
    import bass_rust
    from concourse import mybir

    n = 0
    for fn in nc.m.functions:
        for bb in fn.blocks:
            insts = list(bb.instructions)
            out = []
            changed = False
            for inst in insts:
                si = getattr(inst, "sync_info", None)
                if si is not None and si.on_wait and len(si.on_wait) > _MAXW:
                    waits = list(si.on_wait)
                    rest, keep = waits[:-_MAXW], waits[-_MAXW:]
                    si.on_wait = keep
                    for i in range(0, len(rest), _MAXW):
                        n += 1
                        nop = mybir.InstNoOp(name=f"waitnop-{n}")
                        nop.engine = inst.engine
                        nop.sync_info = bass_rust.SyncInfo(
                            on_wait=rest[i:i + _MAXW], on_update=[])
                        nc.register_instruction(nop, overwrite=True)
                        out.append(nop)
                    changed = True
                out.append(inst)
            if changed:
                bb.instructions = out
    return n


# --------------------------------------------------------------------------
# device program
# --------------------------------------------------------------------------

_PROGRAM_CACHE = {}


def build_program(row_counts, col_counts):
    """row_counts/col_counts: per-class tiles-per-core tuples."""
    import concourse.bass as bass
    import concourse.tile as tile
    from concourse import mybir

    _install_tile_patch()

    key = (tuple(row_counts), tuple(col_counts))
    if key in _PROGRAM_CACHE:
        return _PROGRAM_CACHE[key]

    NRT = sum(row_counts)
    NCT = sum(col_counts)
    nc = bass.Bass("TRN2", target_bir_lowering=False, debug=False,
                   num_devices=NCORES)
    dt = mybir.dt

    def din(name, shape, d):
        return nc.dram_tensor(name, shape, d, kind="ExternalInput").ap()

    lhsx_d = din("lhsx", [36, NRT * 128], dt.bfloat16)
    rhs_row_d = {}
    for ci, cnt in enumerate(row_counts):
        if cnt:
            rhs_row_d[ci] = din(f"rhsrow{ci}",
                                [cnt, 36, ROW_CLASSES[ci]], dt.bfloat16)
    lhsy_d = din("lhsy", [36, NCT * 128], dt.bfloat16)
    rhs_col_d = {}
    for ci, cnt in enumerate(col_counts):
        if cnt:
            rhs_col_d[ci] = din(f"rhscol{ci}",
                                [cnt, 36, COL_CLASSES[ci]], dt.bfloat16)
    bs_d = din("bs", [128, NRT], dt.uint32)
    probsY_d = din("probsY", [128, NCT], dt.float32)
    part_d = nc.dram_tensor("part", [128, 1], dt.float32,
                            kind="ExternalOutput").ap()
    rowmin_d = nc.dram_tensor("rowmin", [128, NRT], dt.float32,
                              kind="ExternalOutput").ap()
    idxg_d = nc.dram_tensor("idxg", [128, NRT], dt.uint32,
                            kind="ExternalOutput").ap()
    colmin_d = nc.dram_tensor("colmin", [128, NCT], dt.float32,
                              kind="ExternalOutput").ap()

    with tile.TileContext(nc) as tc, ExitStack() as ctx:
        resid = ctx.enter_context(tc.tile_pool(name="resid", bufs=1))
        stream = ctx.enter_context(tc.tile_pool(name="stream", bufs=4))
        bstream = ctx.enter_context(tc.tile_pool(name="bstream", bufs=2))
        ps = ctx.enter_context(tc.tile_pool(name="ps", bufs=4, space="PSUM"))
        bands = ctx.enter_context(tc.tile_pool(name="bands", bufs=6))
        bigband = ctx.enter_context(tc.tile_pool(name="bigband", bufs=1))
        small = ctx.enter_context(tc.tile_pool(name="small", bufs=2))

        lhsx = resid.tile([36, NRT * 128], dt.bfloat16)
        nc.sync.dma_start(lhsx[:], lhsx_d[:])
        lhsy = resid.tile([36, NCT * 128], dt.bfloat16)
        nc.sync.dma_start(lhsy[:], lhsy_d[:])
        bs = resid.tile([128, NRT], dt.uint32)
        nc.sync.dma_start(bs[:], bs_d[:])
        probsY = resid.tile([128, NCT], dt.float32)
        nc.sync.dma_start(probsY[:], probsY_d[:])

        rowmax8 = resid.tile([128, NRT * 8], dt.float32)
        colmax8 = resid.tile([128, NCT * 8], dt.float32)
        idx8 = resid.tile([128, NRT * 8], dt.uint32)

        def scan_half(slot8, W, lhs_ap, rhs_dram, idx_slot8):
            """One band scan of width W <= 8192."""
            if W <= 2560:
                ry = stream.tile([36, W], dt.bfloat16, tag=f"rs{W}")
            else:
                ry = bstream.tile([36, W], dt.bfloat16, tag="rsbig")
            nc.sync.dma_start(ry[:], rhs_dram)
            if W <= 2560:
                band = bands.tile([128, W], dt.float32, tag=f"b{W}")
            else:
                band = bigband.tile([128, W], dt.float32, tag="big")
            for c0 in range(0, W, CHUNK):
                cw = min(CHUNK, W - c0)
                pst = ps.tile([128, cw], dt.float32, tag="ps")
                for h0 in range(0, cw, 512):
                    hw = min(512, cw - h0)
                    nc.tensor.matmul(
                        pst[:, h0:h0 + hw],
                        lhs_ap,
                        ry[:, c0 + h0:c0 + h0 + hw],
                        start=True, stop=True,
                    )
                nc.scalar.copy(band[:, c0:c0 + cw], pst[:])
            nc.vector.max(slot8, band[:])
            if idx_slot8 is not None:
                nc.vector.max_index(idx_slot8, slot8, band[:])

        def scan_tile(slot8, W, lhs_ap, rhs_dram, idx_slot8=None):
            """PE matmuls + ACT copy into fp32 bands; DVE max (+max_index).

            Bands wider than 8192 (col side only, no argmin needed) are
            scanned in 8192-halves whose top-8s are max-combined."""
            if W <= 8192:
                scan_half(slot8, W, lhs_ap, rhs_dram, idx_slot8)
                return
            assert idx_slot8 is None
            tmp8 = small.tile([128, 8], dt.float32, tag="tmp8")
            for h in range(W // 8192):
                dst = slot8 if h == 0 else tmp8[:]
                scan_half(dst, 8192, lhs_ap,
                          rhs_dram[:, h * 8192:(h + 1) * 8192], None)
                if h > 0:
                    nc.vector.tensor_tensor(out=slot8, in0=slot8,
                                            in1=tmp8[:],
                                            op=mybir.AluOpType.max)

        # row side; the narrow first class is processed 4 tiles per PSUM
        # tile / ACT copy to amortize per-op overheads
        slot = 0
        for ci, cnt in enumerate(row_counts):
            W = ROW_CLASSES[ci]
            G = max(1, min(8, 1024 // W))
            for j0 in range(0, cnt - cnt % G, G):
                if G == 1:
                    scan_tile(rowmax8[:, slot * 8:(slot + 1) * 8], W,
                              lhsx[:, slot * 128:(slot + 1) * 128],
                              rhs_row_d[ci][j0],
                              idx8[:, slot * 8:(slot + 1) * 8])
                    slot += 1
                    continue
                ry = stream.tile([36, G * W], dt.bfloat16, tag=f"rs{W}g")
                src = rhs_row_d[ci][j0:j0 + G].rearrange("t k w -> k t w")
                nc.sync.dma_start(ry[:].rearrange("k (t w) -> k t w", t=G),
                                  src)
                pst = ps.tile([128, G * W], dt.float32, tag="ps")
                for g in range(G):
                    sl = slot + g
                    nc.tensor.matmul(
                        pst[:, g * W:(g + 1) * W],
                        lhsx[:, sl * 128:(sl + 1) * 128],
                        ry[:, g * W:(g + 1) * W],
                        start=True, stop=True,
                    )
                band = bands.tile([128, G * W], dt.float32, tag=f"b{W}g")
                half = G * W // 2
                nc.scalar.copy(band[:, :half], pst[:, :half])
                nc.scalar.copy(band[:, half:], pst[:, half:])
                for g in range(G):
                    sl = slot + g
                    nc.vector.max(rowmax8[:, sl * 8:(sl + 1) * 8],
                                  band[:, g * W:(g + 1) * W])
                    nc.vector.max_index(idx8[:, sl * 8:(sl + 1) * 8],
                                        rowmax8[:, sl * 8:(sl + 1) * 8],
                                        band[:, g * W:(g + 1) * W])
                slot += G
            for j in range(cnt - cnt % G, cnt):
                scan_tile(rowmax8[:, slot * 8:(slot + 1) * 8], W,
                          lhsx[:, slot * 128:(slot + 1) * 128],
                          rhs_row_d[ci][j],
                          idx8[:, slot * 8:(slot + 1) * 8])
                slot += 1

        # col side
        slot = 0
        for ci, cnt in enumerate(col_counts):
            W = COL_CLASSES[ci]
            for j in range(cnt):
                scan_tile(colmax8[:, slot * 8:(slot + 1) * 8], W,
                          lhsy[:, slot * 128:(slot + 1) * 128],
                          rhs_col_d[ci][j])
                slot += 1

        # endgame
        idx0 = idx8[:].rearrange("p (t e) -> p t e", e=8)[:, :, 0]
        idxg = small.tile([128, NRT], dt.uint32, tag="idxg")
        nc.vector.tensor_tensor(out=idxg[:], in0=idx0, in1=bs[:],
                                op=mybir.AluOpType.add)
        rowmax0 = rowmax8[:].rearrange("p (t e) -> p t e", e=8)[:, :, 0]
        rowmin = small.tile([128, NRT], dt.float32, tag="rowmin")
        nc.vector.tensor_scalar(
            out=rowmin[:], in0=rowmax0, scalar1=-1.0, scalar2=0.0,
            op0=mybir.AluOpType.mult, op1=mybir.AluOpType.max,
        )
        colmax0 = colmax8[:].rearrange("p (t e) -> p t e", e=8)[:, :, 0]
        colmin = small.tile([128, NCT], dt.float32, tag="colmin")
        nc.vector.tensor_scalar(
            out=colmin[:], in0=colmax0, scalar1=-1.0, scalar2=0.0,
            op0=mybir.AluOpType.mult, op1=mybir.AluOpType.max,
        )
        part = small.tile([128, 1], dt.float32, tag="part")
        junk2 = small.tile([128, NCT], dt.float32, tag="junk2")
        nc.vector.scalar_tensor_tensor(
            out=junk2[:], in0=colmin[:], scalar=1.0, in1=probsY[:],
            op0=mybir.AluOpType.mult, op1=mybir.AluOpType.mult,
            accum_out=part[:, 0:1],
        )
        nc.sync.dma_start(part_d[:], part[:])
        nc.sync.dma_start(rowmin_d[:], rowmin[:])
        nc.sync.dma_start(idxg_d[:], idxg[:])
        nc.sync.dma_start(colmin_d[:], colmin[:])

    _split_excess_waits(nc)
    _PROGRAM_CACHE[key] = nc
    return nc


# --------------------------------------------------------------------------
# per-core input assembly
# --------------------------------------------------------------------------

def build_inputs(x, y, p, P=None):
    if P is None:
        P = prep(x, y, p)
    xs, ys, ps = P["xs"], P["ys"], P["ps"]
    row_plan, col_plan = P["row_plan"], P["col_plan"]

    LX, RY = build_lhs_rhs(xs, ys)
    LY, RX = build_lhs_rhs(ys, xs)

    row_counts = tuple(len(row_plan[ci]) // NCORES
                       for ci in range(len(ROW_CLASSES)))
    col_counts = tuple(len(col_plan[ci]) // NCORES
                       for ci in range(len(COL_CLASSES)))

    in_maps = []
    meta = []
    row_groups, col_groups = {}, {}
    for c in range(NCORES):
        lhsx_cols, bs_cols, vmask_cols = [], [], []
        rhs_row = {}
        for ci in range(len(ROW_CLASSES)):
            W = ROW_CLASSES[ci]
            cnt = row_counts[ci]
            if cnt == 0:
                continue
            bandlist = []
            for j in range(cnt):
                mem, s, mask, gid = row_plan[ci][j * NCORES + c]
                lhsx_cols.append(LX[:, mem])
                seg = RY[:, s:s + W]
                if seg.shape[1] < W:
                    seg = np.pad(seg, ((0, 0), (0, W - seg.shape[1])))
                bandlist.append(seg)
                bs_cols.append(np.full(128, s, np.uint32))
                if gid is not None:
                    slot_abs = sum(row_counts[:ci]) + j
                    row_groups.setdefault(gid, {"mask": mask, "mem": mem,
                                                "segs": []})
                    row_groups[gid]["segs"].append((c, slot_abs))
                    mask = np.zeros_like(mask)
                vmask_cols.append(mask)
            rhs_row[f"rhsrow{ci}"] = np.ascontiguousarray(np.stack(bandlist))
        lhsx = np.ascontiguousarray(np.concatenate(lhsx_cols, axis=1))
        bsa = np.stack(bs_cols, axis=1)
        vmask = np.stack(vmask_cols, axis=1).astype(np.float32)

        lhsy_cols, probsY_cols = [], []
        rhs_col = {}
        for ci in range(len(COL_CLASSES)):
            W = COL_CLASSES[ci]
            cnt = col_counts[ci]
            if cnt == 0:
                continue
            bandlist = []
            for j in range(cnt):
                mem, s, mask, gid = col_plan[ci][j * NCORES + c]
                lhsy_cols.append(LY[:, mem])
                seg = RX[:, s:s + W]
                if seg.shape[1] < W:
                    seg = np.pad(seg, ((0, 0), (0, W - seg.shape[1])))
                bandlist.append(seg)
                if gid is not None:
                    slot_abs = sum(col_counts[:ci]) + j
                    col_groups.setdefault(gid, {"mask": mask, "mem": mem,
                                                "segs": []})
                    col_groups[gid]["segs"].append((c, slot_abs))
                    mask = np.zeros_like(mask)
                probsY_cols.append((ps[mem] * mask).astype(np.float32))
            rhs_col[f"rhscol{ci}"] = np.ascontiguousarray(np.stack(bandlist))
        lhsy = np.ascontiguousarray(np.concatenate(lhsy_cols, axis=1))
        probsY = np.stack(probsY_cols, axis=1)

        im = {"lhsx": lhsx, "lhsy": lhsy, "bs": bsa, "probsY": probsY}
        im.update(rhs_row)
        im.update(rhs_col)
        in_maps.append(im)
        meta.append({"vmask": vmask})
    groups = {"row": row_groups, "col": col_groups}
    return in_maps, P, row_counts, col_counts, (meta, groups)


def combine(results, meta_groups, ps):
    """l1 from device partials; l2 = sum ps[argmin]*rowmin*mask on host.

    Wide tiles are scanned as 8 band segments spread across cores; their
    per-segment mins/argmins are min-combined here before the dot."""
    meta, groups = meta_groups
    tot = 0.0
    for r, mt in zip(results, meta):
        tot += float(r["part"].astype(np.float64).sum())
        idx = np.clip(r["idxg"].astype(np.int64), 0, M - 1)
        tot += float((ps[idx].astype(np.float64)
                      * r["rowmin"].astype(np.float64)
                      * mt["vmask"].astype(np.float64)).sum())
    for g in groups["row"].values():
        rm = np.stack([results[c]["rowmin"][:, j] for c, j in g["segs"]])
        ix = np.stack([np.clip(results[c]["idxg"][:, j].astype(np.int64),
                               0, M - 1) for c, j in g["segs"]])
        k = rm.argmin(0)
        rmin = rm[k, np.arange(128)]
        idx = ix[k, np.arange(128)]
        tot += float((ps[idx].astype(np.float64)
                      * rmin.astype(np.float64)
                      * g["mask"].astype(np.float64)).sum())
    for g in groups["col"].values():
        cm = np.stack([results[c]["colmin"][:, j] for c, j in g["segs"]])
        cmin = cm.min(0)
        tot += float((ps[g["mem"]].astype(np.float64)
                      * cmin.astype(np.float64)
                      * g["mask"].astype(np.float64)).sum())
    return np.float32(tot)


# --------------------------------------------------------------------------
# entry point
# --------------------------------------------------------------------------

def kernel(input_vertex_set, sampled_points, probs):
    from concourse.bass_utils import run_bass_kernel_spmd

    x = np.asarray(input_vertex_set, np.float32)
    y = np.asarray(sampled_points, np.float32)
    p = np.asarray(probs, np.float32)

    in_maps, P, row_counts, col_counts, mg = build_inputs(x, y, p)
    nc = build_program(row_counts, col_counts)
    res = run_bass_kernel_spmd(nc, in_maps, list(range(NCORES)))
    return combine(res.results, mg, P["ps"])
